# revision 16
# baseline (speedup 1.0000x reference)
"""AttentionPooling (segment softmax-mean) Trainium2 kernel.

pooled[g] = mean over graph g of softmax_g(score)-weighted x rows, where
score_i = tanh(x_i @ w1 + b1) @ w2 + b2 and graph ids (batch) are sorted.

Strategy: 8 cores, graphs split contiguously and node-balanced. One pass
over x per core with unnormalized e_i = exp(score_i) (scores are O(1), no
overflow). Host supplies x in TWO bf16 layouts (halving HBM traffic and
making every DMA line >=4KB):
  xT  [nsb, 128, 2, 2048]  feature-partitioned (for the MLP matmul)
  xn  [nsb, 128, 16, 256]  node-partitioned    (for the weighted pool)

Per 512-node batch on device (no PE transposes, no DVE reduces):
  PE h-matmul (w1 stationary, xT moving) -> ACT tanh -> PE score matmul
  with h as STATIONARY and w2 moving, so scores land node-partitioned
  [128,4] -> ACT exp -> DVE builds a [128 nodes, 128 graphs] e-weighted
  one-hot stationary -> PE matmul vs natural-x moving accumulates
  pooled[g, d] directly into a persistent PSUM tile across all batches.
e is kept in a resident SBUF strip and exported once; denominators and
the final per-graph scalar normalization are applied host-side.

The Bass program is JIT-specialized per call: graph-slice boundaries from
the actual (sorted) batch vector are baked in as compile-time constants,
so each core gets its own program, built and compiled in parallel.
"""
import numpy as np

N_CORES = 8
D = 256
H = 128
NB = 512            # nodes per compute batch
NCH = NB // 128     # 128-node chunks per batch
SB = 1024           # nodes per DMA super-batch
BPS = SB // NB      # batches per super-batch


def _plan_shards(batch, num_graphs):
    counts = np.bincount(batch, minlength=num_graphs).astype(np.int64)
    starts = np.concatenate([[0], np.cumsum(counts)])  # [B+1]
    n = int(starts[-1])
    cuts = [0]
    for c in range(1, N_CORES):
        target = n * c // N_CORES
        g = int(np.searchsorted(starts, target, side="left"))
        g = max(cuts[-1] + 1, min(g, num_graphs - (N_CORES - c)))
        cuts.append(g)
    cuts.append(num_graphs)
    shards = []
    for c in range(N_CORES):
        g0, g1 = cuts[c], cuts[c + 1]
        n0, n1 = int(starts[g0]), int(starts[g1])
        shards.append(dict(g0=g0, g1=g1, n0=n0, n1=n1,
                           counts=counts[g0:g1],
                           gstarts=starts[g0:g1 + 1] - n0))
    return shards


def _plan_batches(sh):
    """Per batch: {tile t: [(chunk, a, b, gcol), ...]} partition-spans of
    each local graph within each 128-node chunk, grouped by 128-graph
    PSUM tile."""
    nodes = sh["n1"] - sh["n0"]
    nb = (nodes + NB - 1) // NB
    nsb = (nodes + SB - 1) // SB
    G = sh["g1"] - sh["g0"]
    gstarts = sh["gstarts"]
    plans = []
    for b in range(nb):
        lo = b * NB
        groups = {}
        g = max(0, int(np.searchsorted(gstarts, lo, side="right")) - 1)
        for c in range(NCH):
            clo, chi = lo + c * 128, min(lo + (c + 1) * 128, nodes)
            if clo >= chi:
                break
            while g < G and int(gstarts[g]) < chi:
                s, e = max(int(gstarts[g]), clo), min(int(gstarts[g + 1]), chi)
                if e > s:
                    t = g // 128
                    groups.setdefault(t, []).append(
                        (c, s - clo, e - clo, g - t * 128))
                if int(gstarts[g + 1]) <= chi:
                    g += 1
                else:
                    break
        plans.append(groups)
    return nb, nsb, G, plans


def _build_core_program(sh, b2f):
    import concourse.bacc as bacc
    import concourse.mybir as mybir
    import concourse.tile as tile

    nb, nsb, G, plans = _plan_batches(sh)
    npad = nsb * SB
    ntiles = (G + 127) // 128
    assert ntiles <= 4
    f32, bf16 = mybir.dt.float32, mybir.dt.bfloat16
    AF = mybir.ActivationFunctionType

    # per-tile chunk-matmul counts, to place start/stop flags
    mm_total = [0] * ntiles
    nspan = 0
    for groups in plans:
        for t, sp in groups.items():
            mm_total[t] += len({c for (c, a, e, gc) in sp})
            nspan += len(sp)
    mm_seen = [0] * ntiles
    nspan_p = max(nspan, 1)

    nc = bacc.Bacc("TRN2", target_bir_lowering=False, debug=False)
    xT = nc.declare_dram_parameter("xT", [nsb, 128, 2, SB], bf16, isOutput=False)
    xn = nc.declare_dram_parameter("xn", [nsb, 128, SB // 128, D], bf16,
                                   isOutput=False)
    w1_in = nc.declare_dram_parameter("w1", [D, H], bf16, isOutput=False)
    b1_in = nc.declare_dram_parameter("b1", [H, 1], f32, isOutput=False)
    w2_in = nc.declare_dram_parameter("w2", [H, 1], bf16, isOutput=False)
    msk_in = nc.declare_dram_parameter("msk", [128, nspan_p], bf16,
                                       isOutput=False)
    out_p = nc.declare_dram_parameter("pooled", [G, D], f32, isOutput=True)
    e_out = nc.declare_dram_parameter("e", [128, NCH * nb], bf16, isOutput=True)

    with tile.TileContext(nc) as tc:
        with tc.tile_pool(name="const", bufs=1) as const, \
             tc.tile_pool(name="xtp", bufs=4) as xtp, \
             tc.tile_pool(name="xnp", bufs=4) as xnp, \
             tc.tile_pool(name="hp", bufs=4) as hp, \
             tc.tile_pool(name="ep", bufs=6) as ep, \
             tc.tile_pool(name="fin", bufs=1) as fin, \
             tc.tile_pool(name="ps_h", bufs=3, space="PSUM") as ps_h, \
             tc.tile_pool(name="ps_s", bufs=2, space="PSUM") as ps_s, \
             tc.tile_pool(name="ps_p", bufs=1, space="PSUM") as ps_p:

            # ---- constants ----
            w1sb = const.tile([128, 2, H], bf16, tag="w1sb")
            nc.sync.dma_start(out=w1sb,
                              in_=w1_in.rearrange("(f p) h -> p f h", f=2))
            b1col = const.tile([H, 1], f32, tag="b1col")
            nc.sync.dma_start(out=b1col, in_=b1_in[:, :])
            w2sb = const.tile([H, 1], bf16, tag="w2sb")
            nc.sync.dma_start(out=w2sb, in_=w2_in[:, :])
            msk = const.tile([128, nspan_p], bf16, tag="msk")
            nc.sync.dma_start(out=msk, in_=msk_in[:, :])

            # resident e strip: col b*NCH+c holds e for nodes [b*512+c*128+p]
            estore = const.tile([128, NCH * nb], bf16, tag="estore")

            # persistent pooled accumulators [graph, D] per 128-graph tile
            pp = [ps_p.tile([128, D], f32, tag="pp", name=f"pp{t}")
                  for t in range(ntiles)]

            si = 0
            for sb_i in range(nsb):
                xt_t = xtp.tile([128, 2, SB], bf16, tag="xt")
                nc.sync.dma_start(out=xt_t, in_=xT[sb_i])
                xn_t = xnp.tile([128, SB // 128, D], bf16, tag="xn")
                nc.sync.dma_start(out=xn_t, in_=xn[sb_i])

                for bl in range(BPS):
                    b = sb_i * BPS + bl
                    if b >= nb:
                        break
                    groups = plans[b]

                    # h = tanh(x @ w1 + b1), feature-major [H, NB]
                    h_ps = ps_h.tile([H, NB], f32, tag="h")
                    for f in range(2):
                        nc.tensor.matmul(
                            h_ps, w1sb[:, f, :],
                            xt_t[:, f, bl * NB:(bl + 1) * NB],
                            start=(f == 0), stop=(f == 1))
                    h_sb = hp.tile([H, NB], bf16, tag="hsb")
                    nc.scalar.activation(out=h_sb, in_=h_ps, func=AF.Tanh,
                                         bias=b1col, scale=1.0)

                    # scores node-partitioned: lhsT = h chunk, rhs = w2
                    s_ps = ps_s.tile([128, NCH], f32, tag="s")
                    for c in range(NCH):
                        nc.tensor.matmul(
                            s_ps[:, c:c + 1],
                            h_sb[:, c * 128:(c + 1) * 128], w2sb,
                            start=True, stop=True)
                    nc.scalar.activation(
                        out=estore[:, b * NCH:(b + 1) * NCH], in_=s_ps,
                        func=AF.Exp, bias=b2f, scale=1.0)

                    # e-weighted one-hot stationary, pooled matmul
                    for t, sp in sorted(groups.items()):
                        eoh = ep.tile([128, NCH * 128], bf16, tag="eoh",
                                      name=f"eoh{b}_{t}")
                        nc.vector.memset(eoh, 0.0)
                        for (c, a, e, gc) in sp:
                            nc.vector.tensor_mul(
                                out=eoh[:, c * 128 + gc:c * 128 + gc + 1],
                                in0=estore[:, b * NCH + c:b * NCH + c + 1],
                                in1=msk[:, si:si + 1])
                            si += 1
                        for c in sorted({c for (c, a, e, gc) in sp}):
                            mm_seen[t] += 1
                            nc.tensor.matmul(
                                pp[t], eoh[:, c * 128:(c + 1) * 128],
                                xn_t[:, bl * NCH + c, :],
                                start=(mm_seen[t] == 1),
                                stop=(mm_seen[t] == mm_total[t]))

            # ---- finalization ----
            for t in range(ntiles):
                if mm_total[t] == 0:
                    continue
                gw = min(128, G - t * 128)
                o_sb = fin.tile([128, D], f32, tag="osb", name=f"osb{t}")
                nc.vector.tensor_copy(o_sb, pp[t])
                nc.sync.dma_start(out=out_p[t * 128:t * 128 + gw, :],
                                  in_=o_sb[:gw, :])
            nc.sync.dma_start(out=e_out[:, :], in_=estore)

    nc.compile()
    return nc, nb, nsb, G


def _core_in_map(sh, x, w1, b1, w2):
    import ml_dtypes
    bf16 = ml_dtypes.bfloat16
    nodes = sh["n1"] - sh["n0"]
    nsb = (nodes + SB - 1) // SB
    npad = nsb * SB
    xp = np.zeros((npad, D), dtype=np.float32)
    xp[:nodes] = x[sh["n0"]:sh["n1"]]
    xb = xp.astype(bf16)
    # xT[s, p, f, n] = x[s*SB + n, f*128 + p]
    xT = np.ascontiguousarray(
        xb.reshape(nsb, SB, 2, 128).transpose(0, 3, 2, 1))
    # xn[s, p, c, d] = x[s*SB + c*128 + p, d]
    xn = np.ascontiguousarray(
        xb.reshape(nsb, SB // 128, 128, D).transpose(0, 2, 1, 3))
    # span masks, in program emission order (b, t asc, span order)
    nb2, nsb2, G, plans = _plan_batches(sh)
    spans = [s for groups in plans
             for t, sp in sorted(groups.items()) for s in sp]
    mskf = np.zeros((128, max(len(spans), 1)), np.float32)
    for i, (c, a, e, gc) in enumerate(spans):
        mskf[a:e, i] = 1.0
    return {"xT": xT, "xn": xn,
            "w1": np.asarray(w1, np.float32).astype(bf16),
            "b1": np.asarray(b1, np.float32).reshape(H, 1),
            "w2": np.asarray(w2, np.float32).astype(bf16).reshape(H, 1),
            "msk": mskf.astype(bf16)}


def _finalize(sh, res, out):
    """Host: divide pooled sums by (sum_g e) * count_g."""
    nodes = sh["n1"] - sh["n0"]
    nb = (nodes + NB - 1) // NB
    pooled = res["pooled"].astype(np.float64)
    e_lin = res["e"].astype(np.float64).T.reshape(-1)[:nodes]
    gstarts = sh["gstarts"]
    seg_len = np.diff(gstarts)
    denom = np.add.reduceat(e_lin, gstarts[:-1]) if nodes else None
    if (seg_len == 0).any():
        denom = np.where(seg_len == 0, 0.0, denom)
    scale = denom * np.maximum(sh["counts"], 1.0)
    scale = np.where(seg_len == 0, 1.0, scale)
    pooled /= scale[:, None]
    pooled[seg_len == 0] = 0.0
    out[sh["g0"]:sh["g1"]] = pooled.astype(np.float32)


def kernel(x, batch, num_graphs, w1, b1, w2, b2):
    from concourse.bass_utils import run_bass_kernel_spmd

    x = np.asarray(x, dtype=np.float32)
    batch = np.asarray(batch).astype(np.int64)
    B = int(num_graphs)
    b2f = float(np.asarray(b2, dtype=np.float32).reshape(-1)[0])

    shards = _plan_shards(batch, B)
    out = np.zeros((B, D), dtype=np.float32)

    import concurrent.futures as cf

    def build(c):
        sh = shards[c]
        nc, nb, nsb, G = _build_core_program(sh, b2f)
        in_map = _core_in_map(sh, x, w1, b1, w2)
        return c, nc, in_map

    with cf.ThreadPoolExecutor(max_workers=8) as ex:
        built = list(ex.map(build, range(N_CORES)))

    for c, nc, in_map in built:
        res = run_bass_kernel_spmd(nc, [in_map], [0])
        _finalize(shards[c], res.results[0], out)
    return out


# revision 18
# speedup vs baseline: 1.1425x; 1.1425x over previous
"""AttentionPooling (segment softmax-mean) Trainium2 kernel.

pooled[g] = mean over graph g of softmax_g(score)-weighted x rows, where
score_i = tanh(x_i @ w1 + b1) @ w2 + b2 and graph ids (batch) are sorted.

Strategy: 8 cores, graphs split contiguously and node-balanced. One pass
over x per core with unnormalized e_i = exp(score_i) (scores are O(1), no
overflow). Host supplies x in TWO bf16 layouts (halving HBM traffic and
making every DMA line >=4KB):
  xT  [nsb, 128, 2, 2048]  feature-partitioned (for the MLP matmul)
  xn  [nsb, 128, 16, 256]  node-partitioned    (for the weighted pool)

Per 512-node batch on device (no PE transposes, no DVE reduces):
  PE h-matmul (w1 stationary, xT moving) -> ACT tanh -> PE score matmul
  with h as STATIONARY and w2 moving, so scores land node-partitioned
  [128,4] -> ACT exp -> DVE builds a [128 nodes, 128 graphs] e-weighted
  one-hot stationary -> PE matmul vs natural-x moving accumulates
  pooled[g, d] directly into a persistent PSUM tile across all batches.
e is kept in a resident SBUF strip and exported once; denominators and
the final per-graph scalar normalization are applied host-side.

The Bass program is JIT-specialized per call: graph-slice boundaries from
the actual (sorted) batch vector are baked in as compile-time constants,
so each core gets its own program, built and compiled in parallel.
"""
import numpy as np

N_CORES = 8
D = 256
H = 128
NB = 512            # nodes per compute batch
NCH = NB // 128     # 128-node chunks per batch
SB = 2048           # nodes per DMA super-batch
BPS = SB // NB      # batches per super-batch


def _plan_shards(batch, num_graphs):
    counts = np.bincount(batch, minlength=num_graphs).astype(np.int64)
    starts = np.concatenate([[0], np.cumsum(counts)])  # [B+1]
    n = int(starts[-1])
    cuts = [0]
    for c in range(1, N_CORES):
        target = n * c // N_CORES
        g = int(np.searchsorted(starts, target, side="left"))
        g = max(cuts[-1] + 1, min(g, num_graphs - (N_CORES - c)))
        cuts.append(g)
    cuts.append(num_graphs)
    shards = []
    for c in range(N_CORES):
        g0, g1 = cuts[c], cuts[c + 1]
        n0, n1 = int(starts[g0]), int(starts[g1])
        shards.append(dict(g0=g0, g1=g1, n0=n0, n1=n1,
                           counts=counts[g0:g1],
                           gstarts=starts[g0:g1 + 1] - n0))
    return shards


def _plan_batches(sh):
    """Per batch: {tile t: [(chunk, a, b, gcol), ...]} partition-spans of
    each local graph within each 128-node chunk, grouped by 128-graph
    PSUM tile."""
    nodes = sh["n1"] - sh["n0"]
    nb = (nodes + NB - 1) // NB
    nsb = (nodes + SB - 1) // SB
    G = sh["g1"] - sh["g0"]
    gstarts = sh["gstarts"]
    plans = []
    for b in range(nb):
        lo = b * NB
        groups = {}
        g = max(0, int(np.searchsorted(gstarts, lo, side="right")) - 1)
        for c in range(NCH):
            clo, chi = lo + c * 128, min(lo + (c + 1) * 128, nodes)
            if clo >= chi:
                break
            while g < G and int(gstarts[g]) < chi:
                s, e = max(int(gstarts[g]), clo), min(int(gstarts[g + 1]), chi)
                if e > s:
                    t = g // 128
                    groups.setdefault(t, []).append(
                        (c, s - clo, e - clo, g - t * 128))
                if int(gstarts[g + 1]) <= chi:
                    g += 1
                else:
                    break
        plans.append(groups)
    return nb, nsb, G, plans


def _build_core_program(sh, b2f):
    import concourse.bacc as bacc
    import concourse.mybir as mybir
    import concourse.tile as tile

    nb, nsb, G, plans = _plan_batches(sh)
    npad = nsb * SB
    ntiles = (G + 127) // 128
    assert ntiles <= 4
    f32, bf16 = mybir.dt.float32, mybir.dt.bfloat16
    AF = mybir.ActivationFunctionType

    # per-tile chunk-matmul counts, to place start/stop flags
    mm_total = [0] * ntiles
    nspan = 0
    for groups in plans:
        for t, sp in groups.items():
            mm_total[t] += len({c for (c, a, e, gc) in sp})
            nspan += len(sp)
    mm_seen = [0] * ntiles
    nspan_p = max(nspan, 1)

    nc = bacc.Bacc("TRN2", target_bir_lowering=False, debug=False)
    xT = nc.declare_dram_parameter("xT", [nsb, 128, 2, SB], bf16, isOutput=False)
    xn = nc.declare_dram_parameter("xn", [nsb, 128, SB // 128, D], bf16,
                                   isOutput=False)
    w1_in = nc.declare_dram_parameter("w1", [D, H], bf16, isOutput=False)
    b1_in = nc.declare_dram_parameter("b1", [H, 1], f32, isOutput=False)
    w2_in = nc.declare_dram_parameter("w2", [H, 1], bf16, isOutput=False)
    msk_in = nc.declare_dram_parameter("msk", [128, nspan_p], bf16,
                                       isOutput=False)
    out_p = nc.declare_dram_parameter("pooled", [G, D], f32, isOutput=True)
    e_out = nc.declare_dram_parameter("e", [128, NCH * nb], bf16, isOutput=True)

    with tile.TileContext(nc) as tc:
        with tc.tile_pool(name="const", bufs=1) as const, \
             tc.tile_pool(name="xtp", bufs=3) as xtp, \
             tc.tile_pool(name="xnp", bufs=3) as xnp, \
             tc.tile_pool(name="hp", bufs=4) as hp, \
             tc.tile_pool(name="ep", bufs=6) as ep, \
             tc.tile_pool(name="fin", bufs=1) as fin, \
             tc.tile_pool(name="ps_h", bufs=3, space="PSUM") as ps_h, \
             tc.tile_pool(name="ps_s", bufs=2, space="PSUM") as ps_s, \
             tc.tile_pool(name="ps_p", bufs=1, space="PSUM") as ps_p:

            # ---- constants ----
            w1sb = const.tile([128, 2, H], bf16, tag="w1sb")
            nc.sync.dma_start(out=w1sb,
                              in_=w1_in.rearrange("(f p) h -> p f h", f=2))
            b1col = const.tile([H, 1], f32, tag="b1col")
            nc.sync.dma_start(out=b1col, in_=b1_in[:, :])
            w2sb = const.tile([H, 1], bf16, tag="w2sb")
            nc.sync.dma_start(out=w2sb, in_=w2_in[:, :])
            msk = const.tile([128, nspan_p], bf16, tag="msk")
            nc.sync.dma_start(out=msk, in_=msk_in[:, :])

            # resident e strip: col b*NCH+c holds e for nodes [b*512+c*128+p]
            estore = const.tile([128, NCH * nb], bf16, tag="estore")

            # persistent pooled accumulators [graph, D] per 128-graph tile
            pp = [ps_p.tile([128, D], f32, tag="pp", name=f"pp{t}")
                  for t in range(ntiles)]

            si = 0
            for sb_i in range(nsb):
                xt_t = xtp.tile([128, 2, SB], bf16, tag="xt")
                nc.sync.dma_start(out=xt_t, in_=xT[sb_i])
                xn_t = xnp.tile([128, SB // 128, D], bf16, tag="xn")
                nc.sync.dma_start(out=xn_t, in_=xn[sb_i])

                for bl in range(BPS):
                    b = sb_i * BPS + bl
                    if b >= nb:
                        break
                    groups = plans[b]

                    # h = tanh(x @ w1 + b1), feature-major [H, NB]
                    h_ps = ps_h.tile([H, NB], f32, tag="h")
                    for f in range(2):
                        nc.tensor.matmul(
                            h_ps, w1sb[:, f, :],
                            xt_t[:, f, bl * NB:(bl + 1) * NB],
                            start=(f == 0), stop=(f == 1))
                    h_sb = hp.tile([H, NB], bf16, tag="hsb")
                    nc.scalar.activation(out=h_sb, in_=h_ps, func=AF.Tanh,
                                         bias=b1col, scale=1.0)

                    # scores node-partitioned: lhsT = h chunk, rhs = w2
                    s_ps = ps_s.tile([128, NCH], f32, tag="s")
                    for c in range(NCH):
                        nc.tensor.matmul(
                            s_ps[:, c:c + 1],
                            h_sb[:, c * 128:(c + 1) * 128], w2sb,
                            start=True, stop=True)
                    nc.scalar.activation(
                        out=estore[:, b * NCH:(b + 1) * NCH], in_=s_ps,
                        func=AF.Exp, bias=b2f, scale=1.0)

                    # e-weighted one-hot stationary, pooled matmul
                    for t, sp in sorted(groups.items()):
                        eoh = ep.tile([128, NCH * 128], bf16, tag="eoh",
                                      name=f"eoh{b}_{t}")
                        nc.vector.memset(eoh, 0.0)
                        for (c, a, e, gc) in sp:
                            nc.vector.tensor_mul(
                                out=eoh[:, c * 128 + gc:c * 128 + gc + 1],
                                in0=estore[:, b * NCH + c:b * NCH + c + 1],
                                in1=msk[:, si:si + 1])
                            si += 1
                        for c in sorted({c for (c, a, e, gc) in sp}):
                            mm_seen[t] += 1
                            nc.tensor.matmul(
                                pp[t], eoh[:, c * 128:(c + 1) * 128],
                                xn_t[:, bl * NCH + c, :],
                                start=(mm_seen[t] == 1),
                                stop=(mm_seen[t] == mm_total[t]))

            # ---- finalization ----
            for t in range(ntiles):
                if mm_total[t] == 0:
                    continue
                gw = min(128, G - t * 128)
                o_sb = fin.tile([128, D], f32, tag="osb", name=f"osb{t}")
                nc.vector.tensor_copy(o_sb, pp[t])
                nc.sync.dma_start(out=out_p[t * 128:t * 128 + gw, :],
                                  in_=o_sb[:gw, :])
            nc.sync.dma_start(out=e_out[:, :], in_=estore)

    nc.compile()
    return nc, nb, nsb, G


def _core_in_map(sh, x, w1, b1, w2):
    import ml_dtypes
    bf16 = ml_dtypes.bfloat16
    nodes = sh["n1"] - sh["n0"]
    nsb = (nodes + SB - 1) // SB
    npad = nsb * SB
    xp = np.zeros((npad, D), dtype=np.float32)
    xp[:nodes] = x[sh["n0"]:sh["n1"]]
    xb = xp.astype(bf16)
    # xT[s, p, f, n] = x[s*SB + n, f*128 + p]
    xT = np.ascontiguousarray(
        xb.reshape(nsb, SB, 2, 128).transpose(0, 3, 2, 1))
    # xn[s, p, c, d] = x[s*SB + c*128 + p, d]
    xn = np.ascontiguousarray(
        xb.reshape(nsb, SB // 128, 128, D).transpose(0, 2, 1, 3))
    # span masks, in program emission order (b, t asc, span order)
    nb2, nsb2, G, plans = _plan_batches(sh)
    spans = [s for groups in plans
             for t, sp in sorted(groups.items()) for s in sp]
    mskf = np.zeros((128, max(len(spans), 1)), np.float32)
    for i, (c, a, e, gc) in enumerate(spans):
        mskf[a:e, i] = 1.0
    return {"xT": xT, "xn": xn,
            "w1": np.asarray(w1, np.float32).astype(bf16),
            "b1": np.asarray(b1, np.float32).reshape(H, 1),
            "w2": np.asarray(w2, np.float32).astype(bf16).reshape(H, 1),
            "msk": mskf.astype(bf16)}


def _finalize(sh, res, out):
    """Host: divide pooled sums by (sum_g e) * count_g."""
    nodes = sh["n1"] - sh["n0"]
    nb = (nodes + NB - 1) // NB
    pooled = res["pooled"].astype(np.float64)
    e_lin = res["e"].astype(np.float64).T.reshape(-1)[:nodes]
    gstarts = sh["gstarts"]
    seg_len = np.diff(gstarts)
    denom = np.add.reduceat(e_lin, gstarts[:-1]) if nodes else None
    if (seg_len == 0).any():
        denom = np.where(seg_len == 0, 0.0, denom)
    scale = denom * np.maximum(sh["counts"], 1.0)
    scale = np.where(seg_len == 0, 1.0, scale)
    pooled /= scale[:, None]
    pooled[seg_len == 0] = 0.0
    out[sh["g0"]:sh["g1"]] = pooled.astype(np.float32)


def kernel(x, batch, num_graphs, w1, b1, w2, b2):
    from concourse.bass_utils import run_bass_kernel_spmd

    x = np.asarray(x, dtype=np.float32)
    batch = np.asarray(batch).astype(np.int64)
    B = int(num_graphs)
    b2f = float(np.asarray(b2, dtype=np.float32).reshape(-1)[0])

    shards = _plan_shards(batch, B)
    out = np.zeros((B, D), dtype=np.float32)

    import concurrent.futures as cf

    def build(c):
        sh = shards[c]
        nc, nb, nsb, G = _build_core_program(sh, b2f)
        in_map = _core_in_map(sh, x, w1, b1, w2)
        return c, nc, in_map

    with cf.ThreadPoolExecutor(max_workers=8) as ex:
        built = list(ex.map(build, range(N_CORES)))

    for c, nc, in_map in built:
        res = run_bass_kernel_spmd(nc, [in_map], [0])
        _finalize(shards[c], res.results[0], out)
    return out


# revision 20
# speedup vs baseline: 1.2116x; 1.0605x over previous
"""AttentionPooling (segment softmax-mean) Trainium2 kernel.

pooled[g] = mean over graph g of softmax_g(score)-weighted x rows, where
score_i = tanh(x_i @ w1 + b1) @ w2 + b2 and graph ids (batch) are sorted.

Strategy: 8 cores, graphs split contiguously and node-balanced. One pass
over x per core with unnormalized e_i = exp(score_i) (scores are O(1), no
overflow). Host supplies x in TWO bf16 layouts (halving HBM traffic and
making every DMA line >=4KB):
  xT  [nsb, 128, 2, 2048]  feature-partitioned (for the MLP matmul)
  xn  [nsb, 128, 16, 256]  node-partitioned    (for the weighted pool)

Per 512-node batch on device (no PE transposes, no DVE reduces):
  PE h-matmul (w1 stationary, xT moving) -> ACT tanh -> PE score matmul
  with h as STATIONARY and w2 moving, so scores land node-partitioned
  [128,4] -> ACT exp -> DVE builds a [128 nodes, 128 graphs] e-weighted
  one-hot stationary -> PE matmul vs natural-x moving accumulates
  pooled[g, d] directly into a persistent PSUM tile across all batches.
e is kept in a resident SBUF strip and exported once; denominators and
the final per-graph scalar normalization are applied host-side.

The Bass program is JIT-specialized per call: graph-slice boundaries from
the actual (sorted) batch vector are baked in as compile-time constants,
so each core gets its own program, built and compiled in parallel.
"""
import numpy as np

N_CORES = 8
D = 256
H = 128
NB = 512            # nodes per compute batch
NCH = NB // 128     # 128-node chunks per batch
SB = 2048           # nodes per DMA super-batch
BPS = SB // NB      # batches per super-batch


def _plan_shards(batch, num_graphs):
    counts = np.bincount(batch, minlength=num_graphs).astype(np.int64)
    starts = np.concatenate([[0], np.cumsum(counts)])  # [B+1]
    n = int(starts[-1])
    cuts = [0]
    for c in range(1, N_CORES):
        target = n * c // N_CORES
        g = int(np.searchsorted(starts, target, side="left"))
        g = max(cuts[-1] + 1, min(g, num_graphs - (N_CORES - c)))
        cuts.append(g)
    cuts.append(num_graphs)
    shards = []
    for c in range(N_CORES):
        g0, g1 = cuts[c], cuts[c + 1]
        n0, n1 = int(starts[g0]), int(starts[g1])
        shards.append(dict(g0=g0, g1=g1, n0=n0, n1=n1,
                           counts=counts[g0:g1],
                           gstarts=starts[g0:g1 + 1] - n0))
    return shards


def _plan_batches(sh):
    """Per batch: {tile t: [(chunk, a, b, gcol), ...]} partition-spans of
    each local graph within each 128-node chunk, grouped by 128-graph
    PSUM tile."""
    nodes = sh["n1"] - sh["n0"]
    nb = (nodes + NB - 1) // NB
    nsb = (nodes + SB - 1) // SB
    G = sh["g1"] - sh["g0"]
    gstarts = sh["gstarts"]
    plans = []
    for b in range(nb):
        lo = b * NB
        groups = {}
        g = max(0, int(np.searchsorted(gstarts, lo, side="right")) - 1)
        for c in range(NCH):
            clo, chi = lo + c * 128, min(lo + (c + 1) * 128, nodes)
            if clo >= chi:
                break
            while g < G and int(gstarts[g]) < chi:
                s, e = max(int(gstarts[g]), clo), min(int(gstarts[g + 1]), chi)
                if e > s:
                    t = g // 128
                    groups.setdefault(t, []).append(
                        (c, s - clo, e - clo, g - t * 128))
                if int(gstarts[g + 1]) <= chi:
                    g += 1
                else:
                    break
        plans.append(groups)
    return nb, nsb, G, plans


def _build_core_program(sh, b2f):
    import concourse.bacc as bacc
    import concourse.mybir as mybir
    import concourse.tile as tile

    nb, nsb, G, plans = _plan_batches(sh)
    npad = nsb * SB
    ntiles = (G + 127) // 128
    assert ntiles <= 4
    f32, bf16 = mybir.dt.float32, mybir.dt.bfloat16
    AF = mybir.ActivationFunctionType

    # per-tile chunk-matmul counts, to place start/stop flags
    mm_total = [0] * ntiles
    nspan = 0
    for groups in plans:
        for t, sp in groups.items():
            mm_total[t] += len({c for (c, a, e, gc) in sp})
            nspan += len(sp)
    mm_seen = [0] * ntiles
    nspan_p = max(nspan, 1)

    nc = bacc.Bacc("TRN2", target_bir_lowering=False, debug=False)
    xT = nc.declare_dram_parameter("xT", [nsb, 128, 2, SB], bf16, isOutput=False)
    xn = nc.declare_dram_parameter("xn", [nsb, 128, SB // 128, D], bf16,
                                   isOutput=False)
    w1_in = nc.declare_dram_parameter("w1", [D, H], bf16, isOutput=False)
    b1_in = nc.declare_dram_parameter("b1", [H, 1], f32, isOutput=False)
    w2_in = nc.declare_dram_parameter("w2", [H, 1], bf16, isOutput=False)
    msk_in = nc.declare_dram_parameter("msk", [128, nspan_p], bf16,
                                       isOutput=False)
    out_p = nc.declare_dram_parameter("pooled", [G, D], f32, isOutput=True)
    e_out = nc.declare_dram_parameter("e", [128, NCH * nb], bf16, isOutput=True)

    with tile.TileContext(nc) as tc:
        with tc.tile_pool(name="const", bufs=1) as const, \
             tc.tile_pool(name="xtp", bufs=4) as xtp, \
             tc.tile_pool(name="xnp", bufs=4) as xnp, \
             tc.tile_pool(name="hp", bufs=4) as hp, \
             tc.tile_pool(name="ep", bufs=6) as ep, \
             tc.tile_pool(name="fin", bufs=1) as fin, \
             tc.tile_pool(name="ps_h", bufs=3, space="PSUM") as ps_h, \
             tc.tile_pool(name="ps_s", bufs=3, space="PSUM") as ps_s, \
             tc.tile_pool(name="ps_p", bufs=1, space="PSUM") as ps_p:

            # ---- constants ----
            w1sb = const.tile([128, 2, H], bf16, tag="w1sb")
            nc.sync.dma_start(out=w1sb,
                              in_=w1_in.rearrange("(f p) h -> p f h", f=2))
            b1col = const.tile([H, 1], f32, tag="b1col")
            nc.sync.dma_start(out=b1col, in_=b1_in[:, :])
            w2sb = const.tile([H, 1], bf16, tag="w2sb")
            nc.sync.dma_start(out=w2sb, in_=w2_in[:, :])
            msk = const.tile([128, nspan_p], bf16, tag="msk")
            nc.sync.dma_start(out=msk, in_=msk_in[:, :])

            # resident e strip: col b*NCH+c holds e for nodes [b*512+c*128+p]
            estore = const.tile([128, NCH * nb], bf16, tag="estore")

            # persistent pooled accumulators [graph, D] per 128-graph tile
            pp = [ps_p.tile([128, D], f32, tag="pp", name=f"pp{t}")
                  for t in range(ntiles)]

            # Software pipeline: per iteration i, emit stage k of batch
            # i-k so every engine's strict-FIFO queue head is always ready
            # (PE never waits on the ACT->DVE chain of the same batch).
            xt_tiles, xn_tiles = {}, {}
            h_ps_t, h_sb_t, s_ps_t, eoh_t = {}, {}, {}, {}
            si = 0
            LAG = 5

            def st_dma(sb_i):
                xt_t = xtp.tile([128, 2, SB], bf16, tag="xt",
                                name=f"xt{sb_i}")
                nc.sync.dma_start(out=xt_t, in_=xT[sb_i])
                xn_t = xnp.tile([128, SB // 128, D], bf16, tag="xn",
                                name=f"xn{sb_i}")
                nc.sync.dma_start(out=xn_t, in_=xn[sb_i])
                xt_tiles[sb_i], xn_tiles[sb_i] = xt_t, xn_t

            def st_h(b):
                bl = b % BPS
                h_ps = ps_h.tile([H, NB], f32, tag="h", name=f"h{b}")
                for f in range(2):
                    nc.tensor.matmul(
                        h_ps, w1sb[:, f, :],
                        xt_tiles[b // BPS][:, f, bl * NB:(bl + 1) * NB],
                        start=(f == 0), stop=(f == 1))
                h_ps_t[b] = h_ps

            def st_tanh(b):
                h_sb = hp.tile([H, NB], bf16, tag="hsb", name=f"hsb{b}")
                nc.scalar.activation(out=h_sb, in_=h_ps_t.pop(b),
                                     func=AF.Tanh, bias=b1col, scale=1.0)
                h_sb_t[b] = h_sb

            def st_scores(b):
                h_sb = h_sb_t.pop(b)
                s_ps = ps_s.tile([128, NCH], f32, tag="s", name=f"s{b}")
                for c in range(NCH):
                    nc.tensor.matmul(
                        s_ps[:, c:c + 1],
                        h_sb[:, c * 128:(c + 1) * 128], w2sb,
                        start=True, stop=True)
                s_ps_t[b] = s_ps

            def st_exp(b):
                nc.scalar.activation(
                    out=estore[:, b * NCH:(b + 1) * NCH],
                    in_=s_ps_t.pop(b), func=AF.Exp, bias=b2f, scale=1.0)

            def st_eoh(b):
                nonlocal si
                tiles = {}
                for t, sp in sorted(plans[b].items()):
                    eoh = ep.tile([128, NCH * 128], bf16, tag="eoh",
                                  name=f"eoh{b}_{t}")
                    nc.vector.memset(eoh, 0.0)
                    for (c, a, e, gc) in sp:
                        nc.vector.tensor_mul(
                            out=eoh[:, c * 128 + gc:c * 128 + gc + 1],
                            in0=estore[:, b * NCH + c:b * NCH + c + 1],
                            in1=msk[:, si:si + 1])
                        si += 1
                    tiles[t] = eoh
                eoh_t[b] = tiles

            def st_pool(b):
                bl = b % BPS
                tiles = eoh_t.pop(b)
                for t, sp in sorted(plans[b].items()):
                    for c in sorted({c for (c, a, e, gc) in sp}):
                        mm_seen[t] += 1
                        nc.tensor.matmul(
                            pp[t], tiles[t][:, c * 128:(c + 1) * 128],
                            xn_tiles[b // BPS][:, bl * NCH + c, :],
                            start=(mm_seen[t] == 1),
                            stop=(mm_seen[t] == mm_total[t]))

            for i in range(nb + LAG):
                if i < nb and i % BPS == 0:
                    st_dma(i // BPS)
                if i < nb:
                    st_h(i)
                if 0 <= i - 1 < nb:
                    st_tanh(i - 1)
                if 0 <= i - 2 < nb:
                    st_scores(i - 2)
                if 0 <= i - 3 < nb:
                    st_exp(i - 3)
                if 0 <= i - 4 < nb:
                    st_eoh(i - 4)
                if 0 <= i - 5 < nb:
                    st_pool(i - 5)

            # ---- finalization ----
            for t in range(ntiles):
                if mm_total[t] == 0:
                    continue
                gw = min(128, G - t * 128)
                o_sb = fin.tile([128, D], f32, tag="osb", name=f"osb{t}")
                nc.vector.tensor_copy(o_sb, pp[t])
                nc.sync.dma_start(out=out_p[t * 128:t * 128 + gw, :],
                                  in_=o_sb[:gw, :])
            nc.sync.dma_start(out=e_out[:, :], in_=estore)

    nc.compile()
    return nc, nb, nsb, G


def _core_in_map(sh, x, w1, b1, w2):
    import ml_dtypes
    bf16 = ml_dtypes.bfloat16
    nodes = sh["n1"] - sh["n0"]
    nsb = (nodes + SB - 1) // SB
    npad = nsb * SB
    xp = np.zeros((npad, D), dtype=np.float32)
    xp[:nodes] = x[sh["n0"]:sh["n1"]]
    xb = xp.astype(bf16)
    # xT[s, p, f, n] = x[s*SB + n, f*128 + p]
    xT = np.ascontiguousarray(
        xb.reshape(nsb, SB, 2, 128).transpose(0, 3, 2, 1))
    # xn[s, p, c, d] = x[s*SB + c*128 + p, d]
    xn = np.ascontiguousarray(
        xb.reshape(nsb, SB // 128, 128, D).transpose(0, 2, 1, 3))
    # span masks, in program emission order (b, t asc, span order)
    nb2, nsb2, G, plans = _plan_batches(sh)
    spans = [s for groups in plans
             for t, sp in sorted(groups.items()) for s in sp]
    mskf = np.zeros((128, max(len(spans), 1)), np.float32)
    for i, (c, a, e, gc) in enumerate(spans):
        mskf[a:e, i] = 1.0
    return {"xT": xT, "xn": xn,
            "w1": np.asarray(w1, np.float32).astype(bf16),
            "b1": np.asarray(b1, np.float32).reshape(H, 1),
            "w2": np.asarray(w2, np.float32).astype(bf16).reshape(H, 1),
            "msk": mskf.astype(bf16)}


def _finalize(sh, res, out):
    """Host: divide pooled sums by (sum_g e) * count_g."""
    nodes = sh["n1"] - sh["n0"]
    nb = (nodes + NB - 1) // NB
    pooled = res["pooled"].astype(np.float64)
    e_lin = res["e"].astype(np.float64).T.reshape(-1)[:nodes]
    gstarts = sh["gstarts"]
    seg_len = np.diff(gstarts)
    denom = np.add.reduceat(e_lin, gstarts[:-1]) if nodes else None
    if (seg_len == 0).any():
        denom = np.where(seg_len == 0, 0.0, denom)
    scale = denom * np.maximum(sh["counts"], 1.0)
    scale = np.where(seg_len == 0, 1.0, scale)
    pooled /= scale[:, None]
    pooled[seg_len == 0] = 0.0
    out[sh["g0"]:sh["g1"]] = pooled.astype(np.float32)


def kernel(x, batch, num_graphs, w1, b1, w2, b2):
    from concourse.bass_utils import run_bass_kernel_spmd

    x = np.asarray(x, dtype=np.float32)
    batch = np.asarray(batch).astype(np.int64)
    B = int(num_graphs)
    b2f = float(np.asarray(b2, dtype=np.float32).reshape(-1)[0])

    shards = _plan_shards(batch, B)
    out = np.zeros((B, D), dtype=np.float32)

    import concurrent.futures as cf

    def build(c):
        sh = shards[c]
        nc, nb, nsb, G = _build_core_program(sh, b2f)
        in_map = _core_in_map(sh, x, w1, b1, w2)
        return c, nc, in_map

    with cf.ThreadPoolExecutor(max_workers=8) as ex:
        built = list(ex.map(build, range(N_CORES)))

    for c, nc, in_map in built:
        res = run_bass_kernel_spmd(nc, [in_map], [0])
        _finalize(shards[c], res.results[0], out)
    return out
